# revision 1
# baseline (speedup 1.0000x reference)
"""Trainium2 Bass kernel for causal multi-head attention.

Problem: nn_MultiHeadAttention (B=4, N=2048, D=768, H=12, dh=64), fp32 I/O.

Sharding: 8 cores = 4 batches x 2 head-groups (6 heads each).  Each core
computes QKV projections for its 6 heads, causal softmax attention, and a
partial output projection (its heads' rows of Wo).  The two partials per
batch are summed on the host (tensor-parallel reduce), which also adds the
output bias; the kernel writes its partial out^T in bf16 (halves output
DMA traffic) on the GpSimd HWDGE queue so the latency-critical
denominator DMAs on the sync queue are never stuck behind 256KB output
bursts.

Per-core layout strategy (all matmuls in bf16, fp32 accumulate):
  - X^T is prepared host-side: xt[c,p,n] = X[n, 128c+p] (bf16).
  - Q^T, K^T computed as [384, N] (d_out on partitions) directly.
  - V computed in natural [N, 64h] layout, extended with a ones column per
    head so the context matmul also produces the softmax denominators.
  - scores^T tiles [k=128, 2 heads, q=512] in PSUM, exp on ScalarE
    (scale=1/8 fused), causal diag masked by memset + triangle multiply.
  - ctx^T accumulated in PSUM over k chunks; row 64 = sum_k exp (denom).
  - denominators -> reciprocal on a [128,8]-packed tile -> DRAM ->
    partition-broadcast DMA -> per-q inverse scale applied on DVE.
  - out^T = Wo_chunk^T @ cn per 128-row chunk, bf16 PSUM->SBUF copy on
    DVE, DMA out on the GpSimd queue (bias added host-side).
"""

import sys

sys.path.insert(0, "/opt/trn_rl_repo")

import numpy as np
import ml_dtypes

BF16 = ml_dtypes.bfloat16

P = 128
DIN = 768
DH = 384  # per-core output cols of Wq/Wk/Wv (6 heads x 64)
NH = 6  # heads per core
KCH = 6  # d_in chunks (768/128)
QW = 512  # q block width


def build(seq=2048, n_wchunks=3):
    """Build the SPMD single-core program.  seq parameterized for sim tests."""
    import concourse.mybir as mybir
    import concourse.tile as tile
    from concourse import bacc
    from contextlib import ExitStack

    f32 = mybir.dt.float32
    bf16 = mybir.dt.bfloat16
    EXP = mybir.ActivationFunctionType.Exp
    COPY = mybir.ActivationFunctionType.Copy
    MULT = mybir.AluOpType.mult
    ADD = mybir.AluOpType.add

    nqb = seq // QW  # q blocks of 512
    nkc = seq // P  # k chunks of 128
    nqs = seq // P  # out row chunks of 128
    HP = 3  # head pairs

    nc = bacc.Bacc(None, target_bir_lowering=False, debug=False)

    xt_d = nc.dram_tensor("xt", [KCH, P, seq], bf16, kind="ExternalInput")
    wq_d = nc.dram_tensor("wq", [KCH, P, DH], bf16, kind="ExternalInput")
    wk_d = nc.dram_tensor("wk", [KCH, P, DH], bf16, kind="ExternalInput")
    wv_d = nc.dram_tensor("wv", [KCH, P, DH], bf16, kind="ExternalInput")
    wo_d = nc.dram_tensor("wo", [n_wchunks, P, DIN], bf16, kind="ExternalInput")
    tri_d = nc.dram_tensor("tri", [P, P], bf16, kind="ExternalInput")
    # output is stored transposed: out[e_chunk, e_p, q] = full_out[q, 128*e_chunk+e_p]
    out_d = nc.dram_tensor("out", [DIN // P, P, seq], bf16, kind="ExternalOutput")
    inv_d = nc.dram_tensor("inv_scratch", [HP, nqb, 2, QW], f32)

    with tile.TileContext(nc) as tc, ExitStack() as ctx:
        const = ctx.enter_context(tc.tile_pool(name="const", bufs=1))
        io = ctx.enter_context(tc.tile_pool(name="io", bufs=1))
        expp = ctx.enter_context(tc.tile_pool(name="expp", bufs=8))
        crawp = ctx.enter_context(tc.tile_pool(name="crawp", bufs=3))
        smallp = ctx.enter_context(tc.tile_pool(name="smallp", bufs=4))
        invbp = ctx.enter_context(tc.tile_pool(name="invbp", bufs=4))
        outp = ctx.enter_context(tc.tile_pool(name="outp", bufs=3))
        ps = ctx.enter_context(tc.tile_pool(name="ps", bufs=3, space="PSUM"))
        cxps = ctx.enter_context(tc.tile_pool(name="cxps", bufs=1, space="PSUM"))

        # ---------------- persistent inputs ----------------
        xt = const.tile([P, KCH, seq], bf16, name="xt_sb")
        wq = const.tile([P, KCH, DH], bf16, name="wq_sb")
        wk = const.tile([P, KCH, DH], bf16, name="wk_sb")
        wv = const.tile([P, KCH, DH], bf16, name="wv_sb")
        wo = const.tile([P, n_wchunks, DIN], bf16, name="wo_sb")
        tri = const.tile([P, P], bf16, name="tri_sb")
        # inputs: activations stream on the sync HWDGE queue, weights in
        # parallel on the scalar HWDGE queue (ScalarE is idle at startup)
        for c in range(KCH):
            nc.scalar.dma_start(wq[:, c, :], wq_d[c])
            xq = nc.sync if c % 2 == 0 else nc.gpsimd
            xq.dma_start(xt[:, c, :], xt_d[c])
        for c in range(KCH):
            nc.scalar.dma_start(wk[:, c, :], wk_d[c])
        nc.scalar.dma_start(tri[:], tri_d[:])
        for c in range(KCH):
            nc.scalar.dma_start(wv[:, c, :], wv_d[c])
        for c in range(n_wchunks):
            nc.scalar.dma_start(wo[:, c, :], wo_d[c])

        # persistent activations
        qt = io.tile([P, HP, seq], bf16, name="qt_sb")
        kt = io.tile([P, HP, seq], bf16, name="kt_sb")
        vx = io.tile([P, nkc, NH, 65], bf16, name="vx_sb")
        cn = io.tile([P, HP, seq], bf16, name="cn_sb")
        ones = io.tile([65, 64], bf16, name="ones_sb")
        nc.vector.memset(vx[:, :, :, 64:65], 1.0)
        nc.vector.memset(ones[:], 1.0)

        def qk_quarter(pair, quarter):
            """Project one quarter of pair's Q^T/K^T: one weight chunk reused
            across two 512-wide n blocks (kc-outer keeps LDWEIGHTS warm).
            Yields after each matmul so the caller can interleave."""
            wt, dst = (wq, qt) if quarter < 2 else (wk, kt)
            nbs = (0, 1) if quarter % 2 == 0 else (2, 3)
            if nbs[-1] >= nqb:  # small-seq (sim) builds
                nbs = tuple(nb for nb in nbs if nb < nqb)
                if not nbs:
                    return
            pt = ps.tile([P, 2, QW], f32, tag="quad", name="pt")
            for kc in range(KCH):
                for r, nb in enumerate(nbs):
                    nc.tensor.matmul(
                        pt[:, r, :],
                        lhsT=wt[:, kc, pair * P : (pair + 1) * P],
                        rhs=xt[:, kc, nb * QW : (nb + 1) * QW],
                        start=(kc == 0),
                        stop=(kc == KCH - 1),
                    )
                    yield
            for r, nb in enumerate(nbs):
                nc.vector.tensor_copy(dst[:, pair, nb * QW : (nb + 1) * QW], pt[:, r, :])

        def qk_upfront():
            """Pair-0 Q^T and K^T over the first two n blocks, interleaved by
            k chunk so compute follows the xt DMA stream."""
            nbs = tuple(nb for nb in (0, 1) if nb < nqb)
            ptq = ps.tile([P, 2, QW], f32, tag="quad", name="ptq")
            ptk = ps.tile([P, 2, QW], f32, tag="quad", name="ptk")
            for kc in range(KCH):
                for pt, wt in ((ptq, wq), (ptk, wk)):
                    for r, nb in enumerate(nbs):
                        nc.tensor.matmul(
                            pt[:, r, :],
                            lhsT=wt[:, kc, 0:P],
                            rhs=xt[:, kc, nb * QW : (nb + 1) * QW],
                            start=(kc == 0),
                            stop=(kc == KCH - 1),
                        )
            for pt, dst in ((ptq, qt), (ptk, kt)):
                for r, nb in enumerate(nbs):
                    nc.vector.tensor_copy(dst[:, 0, nb * QW : (nb + 1) * QW], pt[:, r, :])

        def v_chunk(nb):
            """Yields after each matmul so the caller can interleave."""
            pt = ps.tile([P, 2, QW], f32, tag="quad", name="pt")
            for kc in range(KCH):
                nc.tensor.matmul(
                    pt[:, 0, :DH],
                    lhsT=xt[:, kc, nb * P : (nb + 1) * P],
                    rhs=wv[:, kc, :],
                    start=(kc == 0),
                    stop=(kc == KCH - 1),
                )
                yield
            nc.vector.tensor_copy(
                vx[:, nb, :, 0:64],
                pt[:, 0, :DH].rearrange("p (h d) -> p h d", d=64),
            )

        def out_proj_t(e, qb):
            """Transposed output projection: out^T[e-chunk, q-block] =
            Wo_chunk^T @ cn.  Yields after each matmul so the caller can
            interleave.  For the final q block (drained after the last exp)
            odd e-chunk copies ride the idle ScalarE so the PSUM->SBUF
            copies pipeline two-wide at the kernel tail."""
            qsl = slice(qb * QW, (qb + 1) * QW)
            op = ps.tile([P, 2, QW], f32, tag="quad", name="op")
            for c in range(n_wchunks):
                nc.tensor.matmul(
                    op[:, 0, :],
                    lhsT=wo[:, c, e * P : (e + 1) * P],
                    rhs=cn[:, c, qsl],
                    start=(c == 0),
                    stop=(c == n_wchunks - 1),
                )
                yield
            ob = outp.tile([P, QW], bf16, name="ob")
            nc.vector.tensor_copy(ob[:], op[:, 0, :])
            nc.gpsimd.dma_start(out_d[e, :, qsl], ob[:])

        class FillQueue:
            """Queue of instruction generators, driven a few matmuls at a
            time from inside the attention loop to fill PE bubbles."""

            def __init__(self):
                self.gens = []  # (label, gen)

            def add(self, gen, label=None):
                self.gens.append((label, gen))

            def step(self, n):
                while n > 0 and self.gens:
                    try:
                        next(self.gens[0][1])
                        n -= 1
                    except StopIteration:
                        self.gens.pop(0)

            def drain_through(self, label):
                while any(lab == label for lab, _ in self.gens):
                    try:
                        next(self.gens[0][1])
                    except StopIteration:
                        self.gens.pop(0)

            def drain(self):
                while self.gens:
                    self.step(1000)

        def attention_qblock(pair, i, fq, drain_label=None, steps=(1, 2), pe_bc=False,
                             defer_finish=False, prev_finish=None):
            """One 512-wide q block of causal attention for a head pair.
            fq: FillQueue driven mid-loop to fill PE bubbles."""
            nj = 4 * i + 4  # active k chunks
            qsl = slice(i * QW, (i + 1) * QW)
            cx = cxps.tile([65, 2, QW], f32, name="cx")
            pend = []  # (j, exp tile) awaiting ctx matmuls

            def ctx_mms(j, et):
                d = j - 4 * i
                off = P * d if d > 0 else 0
                for hh in (0, 1):
                    nc.tensor.matmul(
                        cx[:, hh, off:],
                        lhsT=vx[:, j, 2 * pair + hh, :],
                        rhs=et[:, hh, off:],
                        start=(j == 0),
                        stop=(j == nj - 1),
                    )

            for j in range(nj):
                d = j - 4 * i
                off = P * d if d > 0 else 0
                if j == 1 and prev_finish is not None:
                    prev_finish()
                    prev_finish = None
                sc = ps.tile([P, 2, QW], f32, tag="quad", name="sc")
                for hh in (0, 1):
                    base = 64 * hh
                    nc.tensor.matmul(
                        sc[:, hh, off:],
                        lhsT=kt[base : base + 64, pair, j * P : (j + 1) * P],
                        rhs=qt[base : base + 64, pair, i * QW + off : (i + 1) * QW],
                        start=True,
                        stop=True,
                        tile_position=(base, 0),
                    )
                et = expp.tile([P, 2, QW], bf16, name="et")
                if off:
                    nc.scalar.activation(
                        et[:, :, off:], sc[:, :, off:], EXP, scale=0.125
                    )
                else:
                    nc.scalar.activation(et[:], sc[:], EXP, scale=0.125)
                if d >= 0:
                    for hh in (0, 1):
                        if off:
                            nc.vector.memset(et[:, hh, 0:off], 0.0)
                        nc.vector.tensor_mul(
                            et[:, hh, off : off + P], et[:, hh, off : off + P], tri[:]
                        )
                if len(pend) >= 2:
                    ctx_mms(*pend.pop(0))
                pend.append((j, et))
                fq.step(steps[1] if d >= 0 else steps[0])
            for args in pend:
                ctx_mms(*args)
            if drain_label is not None:
                fq.drain_through(drain_label)

            # ctx + denominators out of PSUM
            cr = crawp.tile([65, 2, QW], f32, name="cr")
            nc.vector.tensor_copy(cr[:], cx[:])

            # denominators -> packed reciprocal -> broadcast
            sp = smallp.tile([P, 8], f32, tag="sp", name="sp")
            nc.sync.dma_start(sp[:], cr[64:65, :, :])
            if pe_bc:
                # Short chain for blocks whose tail is exposed at a drain
                # point: gather the inverses back to a row and broadcast
                # with a K=1 ones matmul into the just-read cx rows -- no
                # DRAM roundtrip, no 64-descriptor broadcast storm.
                ipb = smallp.tile([P, 8], bf16, tag="ip", name="ipb")
                with nc.allow_low_precision(reason="inv denom broadcast bf16"):
                    nc.vector.reciprocal(ipb[:], sp[:])
                rb = smallp.tile([65, 2, QW], bf16, tag="rb", name="rb")
                nc.sync.dma_start(rb[64:65, :, :], ipb[:])

                def finish():
                    # PE broadcast + normalize; deferrable into the next
                    # block so the K=1 matmul's unpack-DMA wait never
                    # blocks the next block's scores in the in-order queue.
                    for hh in (0, 1):
                        nc.tensor.matmul(
                            cx[0:64, hh, :],
                            lhsT=ones[64:65, :],
                            rhs=rb[64:65, hh, :],
                            start=True,
                            stop=True,
                        )
                    tt = smallp.tile([64, QW], bf16, tag="tt", name="tt")
                    nc.vector.tensor_mul(tt[:], cr[0:64, 1, :], cx[0:64, 1, :])
                    nc.sync.dma_start(cn[64:128, pair, qsl], tt[:])
                    nc.vector.tensor_mul(
                        cn[0:64, pair, qsl], cr[0:64, 0, :], cx[0:64, 0, :]
                    )

                if defer_finish:
                    return finish
                finish()
                return
            ip = smallp.tile([P, 8], f32, tag="ip", name="ip")
            nc.vector.reciprocal(ip[:], sp[:])
            nc.sync.dma_start(inv_d[pair, i], ip[:])

            for hh in (0, 1):
                ib = invbp.tile([64, QW], f32, name="ib")
                nc.sync.dma_start(
                    ib[:], inv_d[pair, i, hh : hh + 1, :].broadcast_to((64, QW))
                )
                if hh == 0:
                    nc.vector.tensor_mul(cn[0:64, pair, qsl], cr[0:64, 0, :], ib[:])
                else:
                    tt = smallp.tile([64, QW], bf16, tag="tt", name="tt")
                    nc.vector.tensor_mul(tt[:], cr[0:64, 1, :], ib[:])
                    nc.sync.dma_start(cn[64:128, pair, qsl], tt[:])

        # ---------------- schedule ----------------
        # Dense upfront phases (keeps the PE clock-gate warm), then
        # attention with later pairs' projections / the output projection
        # interleaved as fine-grained fill work.
        for quarter in range(4):
            for _ in qk_quarter(0, quarter):
                pass
        for nb in range(nkc):
            for _ in v_chunk(nb):
                pass
        carry = [None]
        for pair in range(HP):
            last_pair = pair == HP - 1
            fq = FillQueue()
            if not last_pair:
                for quarter in range(4):
                    fq.add(qk_quarter(pair + 1, quarter), f"qk{pair + 1}")
            for i in range(nqb):
                # the last block of pairs 1 and 2 has its tail chain exposed
                # at a fill-drain point: use the short PE-broadcast tail
                # there.  Pair 1's is additionally deferred into pair 2's
                # first block so its broadcast never stalls the transition.
                pe_bc = i == nqb - 1 and pair >= 1
                r = attention_qblock(
                    pair, i, fq, steps=(1, 2), pe_bc=pe_bc,
                    defer_finish=(pe_bc and pair == 1),
                    prev_finish=carry[0] if (pair == 2 and i == 0) else None,
                )
                if pair == 2 and i == 0:
                    carry[0] = None
                if r is not None:
                    carry[0] = r
                if last_pair:
                    for e in range(DIN // P):
                        fq.add(out_proj_t(e, i), "op")
            fq.drain()

    nc.compile()
    return nc


def make_in_maps(X, Wq, Wk, Wv, Wo, bo=None, seq=2048):
    """Shard full inputs into the 8 per-core input maps."""
    X = np.asarray(X, np.float32)
    Wq = np.asarray(Wq, np.float32)
    Wk = np.asarray(Wk, np.float32)
    Wv = np.asarray(Wv, np.float32)
    Wo = np.asarray(Wo, np.float32)
    bo = np.asarray(bo, np.float32)

    tri = np.triu(np.ones((P, P), np.float32)).astype(BF16)

    in_maps = []
    for b in range(X.shape[0]):
        xt = np.ascontiguousarray(X[b].T).astype(BF16).reshape(KCH, P, seq)
        for hg in range(2):
            sl = slice(hg * DH, (hg + 1) * DH)
            in_maps.append(
                {
                    "xt": xt,
                    "wq": np.ascontiguousarray(Wq[:, sl]).astype(BF16).reshape(KCH, P, DH),
                    "wk": np.ascontiguousarray(Wk[:, sl]).astype(BF16).reshape(KCH, P, DH),
                    "wv": np.ascontiguousarray(Wv[:, sl]).astype(BF16).reshape(KCH, P, DH),
                    "wo": np.ascontiguousarray(Wo[sl, :]).astype(BF16).reshape(3, P, DIN),
                    "tri": tri,
                }
            )
    return in_maps


_built = None


def _get_built():
    global _built
    if _built is None:
        _built = build()
    return _built


def run(inputs, trace=False):
    from concourse.bass_utils import run_bass_kernel_spmd

    nc = _get_built()
    in_maps = make_in_maps(**inputs)
    res = run_bass_kernel_spmd(nc, in_maps, list(range(8)), trace=trace)
    # per-core output is stored transposed as [6, 128, seq] = out.T chunked
    parts = [
        np.asarray(r["out"]).astype(np.float32).reshape(DIN, -1).T
        for r in res.results
    ]
    bo = np.asarray(inputs["bo"], np.float32)
    out = np.stack(
        [parts[2 * b] + parts[2 * b + 1] + bo for b in range(len(parts) // 2)]
    )
    return out, res


def kernel(X, Wq, Wk, Wv, Wo, bo):
    out, _ = run(dict(X=X, Wq=Wq, Wk=Wk, Wv=Wv, Wo=Wo, bo=bo))
    return out



# revision 8
# speedup vs baseline: 1.0936x; 1.0936x over previous
"""Trainium2 Bass kernel for causal multi-head attention.

Problem: nn_MultiHeadAttention (B=4, N=2048, D=768, H=12, dh=64), fp32 I/O.

Sharding: 8 cores = 4 batches x 2 head-groups (6 heads each).  Each core
computes QKV projections for its 6 heads, causal softmax attention, and a
partial output projection (its heads' rows of Wo).  The two partials per
batch are summed on the host (tensor-parallel reduce), which also adds the
output bias; the kernel writes its partial out^T in bf16 (halves output
DMA traffic).

Per-core layout strategy (all matmuls in bf16, fp32 accumulate):
  - X^T is prepared host-side: xt[c,p,n] = X[n, 128c+p] (bf16), streamed
    as 12 half-chunks round-robin over the sync/vector/gpsimd/tensor DMA
    queues so the first QK matmuls start ~3us in; a short burst of warmup
    matmuls keeps the PE HAM clock-gate warm through the load.
  - Q^T, K^T computed as [384, N] (d_out on partitions); pair-0's first
    half is kc-interleaved with the xt DMA stream, later pairs run as
    fill work inside the attention loop.
  - V computed in natural [N, 64h] layout, extended with a ones column per
    head so the context matmul also produces the softmax denominators.
  - scores^T tiles [k=128, 2 heads, q=512] in PSUM (two row-tiled
    concurrent matmuls), exp on ScalarE (scale=1/8 fused), causal diagonal
    masked by a triangle multiply.
  - ctx^T accumulated in PSUM over k chunks; row 64 = sum_k exp (denom).
  - denominators -> packed [128,8] reciprocal -> DRAM -> partition-
    broadcast DMAs riding the vector/gpsimd queues (keeps the sync queue
    shallow) -> per-q inverse scale applied on DVE.
  - out^T = Wo_chunk^T @ cn per 128-row chunk; emission of each q-block's
    output projection is delayed by one attention block so its pair-2
    matmul never head-of-line-blocks the in-order PE queue; the final
    q-block runs a two-phase projection (pairs 0/1 accumulate while the
    last denominator chain resolves, pair 2 finishes after) with the
    PSUM->SBUF casts split across DVE/ScalarE and the output DMAs spread
    over three queues.
"""

import sys

sys.path.insert(0, "/opt/trn_rl_repo")

import numpy as np
import ml_dtypes

BF16 = ml_dtypes.bfloat16

P = 128
DIN = 768
DH = 384  # per-core output cols of Wq/Wk/Wv (6 heads x 64)
NH = 6  # heads per core
KCH = 6  # d_in chunks (768/128)
QW = 512  # q block width


def build(seq=2048, n_wchunks=3):
    """Build the SPMD single-core program.  seq parameterized for sim tests."""
    import concourse.mybir as mybir
    import concourse.tile as tile
    from concourse import bacc
    from contextlib import ExitStack

    f32 = mybir.dt.float32
    bf16 = mybir.dt.bfloat16
    EXP = mybir.ActivationFunctionType.Exp
    COPY = mybir.ActivationFunctionType.Copy

    nqb = seq // QW  # q blocks of 512
    nkc = seq // P  # k chunks of 128
    HP = 3  # head pairs
    HSEQ = seq // 2

    nc = bacc.Bacc(None, target_bir_lowering=False, debug=False)

    xt_d = nc.dram_tensor("xt", [KCH, P, seq], bf16, kind="ExternalInput")
    wq_d = nc.dram_tensor("wq", [KCH, P, DH], bf16, kind="ExternalInput")
    wk_d = nc.dram_tensor("wk", [KCH, P, DH], bf16, kind="ExternalInput")
    wv_d = nc.dram_tensor("wv", [KCH, P, DH], bf16, kind="ExternalInput")
    wo_d = nc.dram_tensor("wo", [n_wchunks, P, DIN], bf16, kind="ExternalInput")
    tri_d = nc.dram_tensor("tri", [P, P], bf16, kind="ExternalInput")
    # output is stored transposed: out[e_chunk, e_p, q] = full_out[q, 128*e_chunk+e_p]
    out_d = nc.dram_tensor("out", [DIN // P, P, seq], bf16, kind="ExternalOutput")
    inv_d = nc.dram_tensor("inv_scratch", [HP, nqb, 2, QW], f32)

    with tile.TileContext(nc) as tc, ExitStack() as ctx:
        const = ctx.enter_context(tc.tile_pool(name="const", bufs=1))
        io = ctx.enter_context(tc.tile_pool(name="io", bufs=1))
        expp = ctx.enter_context(tc.tile_pool(name="expp", bufs=8))
        crawp = ctx.enter_context(tc.tile_pool(name="crawp", bufs=3))
        smallp = ctx.enter_context(tc.tile_pool(name="smallp", bufs=4))
        invbp = ctx.enter_context(tc.tile_pool(name="invbp", bufs=4))
        outp = ctx.enter_context(tc.tile_pool(name="outp", bufs=3))
        ps = ctx.enter_context(tc.tile_pool(name="ps", bufs=3, space="PSUM"))
        cxps = ctx.enter_context(tc.tile_pool(name="cxps", bufs=1, space="PSUM"))

        # ---------------- persistent inputs ----------------
        xt = const.tile([P, KCH, seq], bf16, name="xt_sb")
        wq = const.tile([P, KCH, DH], bf16, name="wq_sb")
        wk = const.tile([P, KCH, DH], bf16, name="wk_sb")
        wv = const.tile([P, KCH, DH], bf16, name="wv_sb")
        wo = const.tile([P, n_wchunks, DIN], bf16, name="wo_sb")
        tri = const.tile([P, P], bf16, name="tri_sb")

        # tri leads on sync (warmup matmul source), then xt streams as
        # half-chunks: sync carries chunks 0-2, gpsimd 3-5, first halves
        # before second halves so the upfront QK matmuls (which consume
        # first halves in arrival order) start as soon as possible.
        # pair-0 weight slices lead on the scalar queue.
        nc.sync.dma_start(tri[:], tri_d[:])
        for h in range(2):
            sl = slice(h * HSEQ, (h + 1) * HSEQ)
            for c in range(KCH // 2):
                nc.sync.dma_start(xt[:, c, sl], xt_d[c][:, sl])
                nc.gpsimd.dma_start(
                    xt[:, c + KCH // 2, sl], xt_d[c + KCH // 2][:, sl]
                )
        nc.scalar.dma_start(
            wq[:, :, 0:P], wq_d[:, :, 0:P].rearrange("c p x -> p c x")
        )
        nc.scalar.dma_start(
            wk[:, :, 0:P], wk_d[:, :, 0:P].rearrange("c p x -> p c x")
        )
        for c in range(KCH):
            nc.scalar.dma_start(wv[:, c, :], wv_d[c])
        nc.scalar.dma_start(
            wq[:, :, P:DH], wq_d[:, :, P:DH].rearrange("c p x -> p c x")
        )
        nc.scalar.dma_start(
            wk[:, :, P:DH], wk_d[:, :, P:DH].rearrange("c p x -> p c x")
        )
        for c in range(n_wchunks):
            nc.scalar.dma_start(wo[:, c, :], wo_d[c])

        # persistent activations
        qt = io.tile([P, HP, seq], bf16, name="qt_sb")
        kt = io.tile([P, HP, seq], bf16, name="kt_sb")
        vx = io.tile([P, nkc, NH, 65], bf16, name="vx_sb")
        cn = io.tile([P, HP, seq], bf16, name="cn_sb")
        ones = io.tile([65, 64], bf16, name="ones_sb")
        nc.vector.memset(vx[:, :, :, 64:65], 1.0)
        nc.vector.memset(ones[:], 1.0)

        # ---------------- warmup: keep HAM busy during the load ----------
        warm = ps.tile([P, 2, QW], f32, tag="quad", name="warm")
        for _ in range(10):
            nc.tensor.matmul(
                warm[:, 0, 0:P],
                lhsT=tri[:],
                rhs=tri[:],
                start=True,
                stop=True,
            )

        # xt half-chunks arrive interleaved across the two queues: chunk
        # order 0,3,1,4,2,5 — consume in that order.
        KC_ORDER = [c for pair_ in zip(range(KCH // 2), range(KCH // 2, KCH)) for c in pair_]

        def qk_upfront(nbs):
            """Pair-0 Q^T and K^T over two n blocks, kc-interleaved so the
            matmul stream follows the xt DMA arrivals."""
            nbs = tuple(nb for nb in nbs if nb < nqb)
            if not nbs:
                return
            ptq = ps.tile([P, 2, QW], f32, tag="quad", name="ptq")
            ptk = ps.tile([P, 2, QW], f32, tag="quad", name="ptk")
            for ki, kc in enumerate(KC_ORDER):
                for pt, wt in ((ptq, wq), (ptk, wk)):
                    for r, nb in enumerate(nbs):
                        nc.tensor.matmul(
                            pt[:, r, :],
                            lhsT=wt[:, kc, 0:P],
                            rhs=xt[:, kc, nb * QW : (nb + 1) * QW],
                            start=(ki == 0),
                            stop=(ki == KCH - 1),
                        )
            for pt, dst in ((ptq, qt), (ptk, kt)):
                for r, nb in enumerate(nbs):
                    nc.vector.tensor_copy(dst[:, 0, nb * QW : (nb + 1) * QW], pt[:, r, :])

        def qk_quarter(pair, quarter):
            """Project one quarter of pair's Q^T/K^T: one weight chunk reused
            across two 512-wide n blocks (kc-outer keeps LDWEIGHTS warm).
            Yields after each matmul so the caller can interleave."""
            wt, dst = (wq, qt) if quarter < 2 else (wk, kt)
            nbs = (0, 1) if quarter % 2 == 0 else (2, 3)
            nbs = tuple(nb for nb in nbs if nb < nqb)
            if not nbs:
                return
            pt = ps.tile([P, 2, QW], f32, tag="quad", name="pt")
            for kc in range(KCH):
                for r, nb in enumerate(nbs):
                    nc.tensor.matmul(
                        pt[:, r, :],
                        lhsT=wt[:, kc, pair * P : (pair + 1) * P],
                        rhs=xt[:, kc, nb * QW : (nb + 1) * QW],
                        start=(kc == 0),
                        stop=(kc == KCH - 1),
                    )
                    yield
            for r, nb in enumerate(nbs):
                nc.vector.tensor_copy(dst[:, pair, nb * QW : (nb + 1) * QW], pt[:, r, :])

        def v_chunk(nb):
            """Yields after each matmul so the caller can interleave."""
            pt = ps.tile([P, 2, QW], f32, tag="quad", name="pt")
            for kc in range(KCH):
                nc.tensor.matmul(
                    pt[:, 0, :DH],
                    lhsT=xt[:, kc, nb * P : (nb + 1) * P],
                    rhs=wv[:, kc, :],
                    start=(kc == 0),
                    stop=(kc == KCH - 1),
                )
                yield
            nc.vector.tensor_copy(
                vx[:, nb, :, 0:64],
                pt[:, 0, :DH].rearrange("p (h d) -> p h d", d=64),
            )

        def out_proj_t(e, qb):
            """Transposed output projection: out^T[e-chunk, q-block] =
            Wo_chunk^T @ cn.  Yields after each matmul so the caller can
            interleave; output DMAs alternate queues."""
            qsl = slice(qb * QW, (qb + 1) * QW)
            op = ps.tile([P, 2, QW], f32, tag="quad", name="op")
            for c in range(n_wchunks):
                nc.tensor.matmul(
                    op[:, 0, :],
                    lhsT=wo[:, c, e * P : (e + 1) * P],
                    rhs=cn[:, c, qsl],
                    start=(c == 0),
                    stop=(c == n_wchunks - 1),
                )
                yield
            ob = outp.tile([P, QW], bf16, name="ob")
            nc.vector.tensor_copy(ob[:], op[:, 0, :])
            (nc.gpsimd if e % 2 else nc.sync).dma_start(out_d[e, :, qsl], ob[:])

        class FillQueue:
            """Queue of instruction generators, driven a few matmuls at a
            time from inside the attention loop to fill PE bubbles."""

            def __init__(self):
                self.gens = []

            def add(self, gen):
                self.gens.append(gen)

            def step(self, n):
                while n > 0 and self.gens:
                    try:
                        next(self.gens[0])
                        n -= 1
                    except StopIteration:
                        self.gens.pop(0)

            def drain(self):
                while self.gens:
                    self.step(1000)

        def attention_qblock(pair, i, fq, steps=(1, 2), last=False):
            """One 512-wide q block of causal attention for a head pair.
            fq: FillQueue driven mid-loop to fill PE bubbles.  Returns a
            deferred finish closure when `last` (the kernel tail overlaps
            it with the final output projection)."""
            nj = 4 * i + 4  # active k chunks
            qsl = slice(i * QW, (i + 1) * QW)
            cx = cxps.tile([65, 2, QW], f32, name="cx")
            pend = []  # (j, exp tile) awaiting ctx matmuls

            def ctx_mms(j, et):
                d = j - 4 * i
                off = P * d if d > 0 else 0
                for hh in (0, 1):
                    nc.tensor.matmul(
                        cx[:, hh, off:],
                        lhsT=vx[:, j, 2 * pair + hh, :],
                        rhs=et[:, hh, off:],
                        start=(j == 0),
                        stop=(j == nj - 1),
                    )

            for j in range(nj):
                d = j - 4 * i
                off = P * d if d > 0 else 0
                sc = ps.tile([P, 2, QW], f32, tag="quad", name="sc")
                for hh in (0, 1):
                    base = 64 * hh
                    nc.tensor.matmul(
                        sc[:, hh, off:],
                        lhsT=kt[base : base + 64, pair, j * P : (j + 1) * P],
                        rhs=qt[base : base + 64, pair, i * QW + off : (i + 1) * QW],
                        start=True,
                        stop=True,
                        tile_position=(base, 0),
                    )
                et = expp.tile([P, 2, QW], bf16, name="et")
                if off:
                    nc.scalar.activation(
                        et[:, :, off:], sc[:, :, off:], EXP, scale=0.125
                    )
                else:
                    nc.scalar.activation(et[:], sc[:], EXP, scale=0.125)
                if d >= 0:
                    for hh in (0, 1):
                        nc.vector.tensor_mul(
                            et[:, hh, off : off + P], et[:, hh, off : off + P], tri[:]
                        )
                if len(pend) >= 2:
                    ctx_mms(*pend.pop(0))
                pend.append((j, et))
                fq.step(steps[1] if d >= 0 else steps[0])
            for args in pend:
                ctx_mms(*args)

            # ctx + denominators out of PSUM
            cr = crawp.tile([65, 2, QW], f32, name="cr")
            nc.vector.tensor_copy(cr[:], cx[:])

            # denominators -> packed reciprocal
            sp = smallp.tile([P, 8], f32, tag="sp", name="sp")
            nc.sync.dma_start(sp[:], cr[64:65, :, :])
            if last:
                # Final block: short on-chip chain; the PE ones-broadcast +
                # normalize is deferred so the tail output projection's
                # pair-0/1 matmuls cover the reciprocal round trip.
                ipb = smallp.tile([P, 8], bf16, tag="ip", name="ipb")
                with nc.allow_low_precision(reason="inv denom broadcast bf16"):
                    nc.vector.reciprocal(ipb[:], sp[:])
                rb = smallp.tile([65, 2, QW], bf16, tag="rb", name="rb")
                nc.sync.dma_start(rb[64:65, :, :], ipb[:])

                def finish():
                    for hh in (0, 1):
                        nc.tensor.matmul(
                            cx[0:64, hh, :],
                            lhsT=ones[64:65, :],
                            rhs=rb[64:65, hh, :],
                            start=True,
                            stop=True,
                        )
                    tt = smallp.tile([64, QW], bf16, tag="tt", name="tt")
                    nc.vector.tensor_mul(tt[:], cr[0:64, 1, :], cx[0:64, 1, :])
                    nc.gpsimd.dma_start(cn[64:128, pair, qsl], tt[:])
                    nc.vector.tensor_mul(
                        cn[0:64, pair, qsl], cr[0:64, 0, :], cx[0:64, 0, :]
                    )

                return finish

            ip = smallp.tile([P, 8], f32, tag="ip", name="ip")
            nc.vector.reciprocal(ip[:], sp[:])
            nc.sync.dma_start(inv_d[pair, i], ip[:])

            # inverse broadcasts + normalize; the two broadcasts split
            # across the sync and gpsimd queues.
            for hh in (0, 1):
                ib = invbp.tile([64, QW], f32, name="ib")
                (nc.sync if hh == 0 else nc.gpsimd).dma_start(
                    ib[:], inv_d[pair, i, hh : hh + 1, :].broadcast_to((64, QW))
                )
                if hh == 0:
                    nc.vector.tensor_mul(cn[0:64, pair, qsl], cr[0:64, 0, :], ib[:])
                else:
                    tt = smallp.tile([64, QW], bf16, tag="tt", name="tt")
                    nc.vector.tensor_mul(tt[:], cr[0:64, 1, :], ib[:])
                    (nc.sync if i % 2 else nc.gpsimd).dma_start(
                        cn[64:128, pair, qsl], tt[:]
                    )
            return None

        # ---------------- schedule ----------------
        # DMA-paced upfront QK for pair 0, then dense quarters + V, then
        # attention with later pairs' projections / the output projection
        # interleaved as fine-grained fill work.  The fill queue persists
        # across pair boundaries (no drain bursts).
        qk_upfront((0, 1))
        qk_upfront((2, 3))
        for nb in range(nkc):
            for _ in v_chunk(nb):
                pass
        fq = FillQueue()
        finish_last = None
        for pair in range(HP):
            if pair < HP - 1:
                for quarter in range(4):
                    fq.add(qk_quarter(pair + 1, quarter))
            for i in range(nqb):
                last = pair == HP - 1 and i == nqb - 1
                r = attention_qblock(
                    pair, i, fq, steps=(1, 2) if pair < HP - 1 else (2, 2), last=last
                )
                if r is not None:
                    finish_last = r
                # delay each q block's output projection by one block so its
                # pair-2 matmul never enters the PE queue before the block's
                # normalize chain is emitted and well underway.
                if pair == HP - 1 and i >= 1:
                    for e in range(DIN // P):
                        fq.add(out_proj_t(e, i - 1))

        # ---------------- tail: final q block's output projection --------
        qb = nqb - 1
        qsl = slice(qb * QW, (qb + 1) * QW)
        ne = DIN // P
        fq.drain()
        # phase A: Wo pairs 0/1 accumulate into three double-slot PSUM
        # tiles while the deferred denominator chain resolves off-PE.
        ops = [
            ps.tile([P, 2, QW], f32, tag="quad", name=f"opt{t}")
            for t in range((ne + 1) // 2)
        ]
        for c in range(n_wchunks - 1):
            for e in range(ne):
                nc.tensor.matmul(
                    ops[e // 2][:, e % 2, :],
                    lhsT=wo[:, c, e * P : (e + 1) * P],
                    rhs=cn[:, c, qsl],
                    start=(c == 0),
                    stop=False,
                )
        if finish_last is not None:
            finish_last()
        # phase B: pair-2 contribution, then casts split across DVE and
        # ScalarE and output DMAs spread over three queues.
        for e in range(ne):
            nc.tensor.matmul(
                ops[e // 2][:, e % 2, :],
                lhsT=wo[:, n_wchunks - 1, e * P : (e + 1) * P],
                rhs=cn[:, n_wchunks - 1, qsl],
                start=False,
                stop=True,
            )
        for t in range((ne + 1) // 2):
            ob2 = outp.tile([P, 2, QW], bf16, name="ob2")
            if t % 2 == 1:
                nc.scalar.activation(ob2[:], ops[t][:], COPY)
            else:
                nc.vector.tensor_copy(ob2[:], ops[t][:])
            for k in range(2):
                e = 2 * t + k
                if e < ne:
                    [nc.gpsimd, nc.sync, nc.scalar][e % 3].dma_start(
                        out_d[e, :, qsl], ob2[:, k, :]
                    )

    nc.compile()
    return nc


def make_in_maps(X, Wq, Wk, Wv, Wo, bo=None, seq=2048):
    """Shard full inputs into the 8 per-core input maps."""
    X = np.asarray(X, np.float32)
    Wq = np.asarray(Wq, np.float32)
    Wk = np.asarray(Wk, np.float32)
    Wv = np.asarray(Wv, np.float32)
    Wo = np.asarray(Wo, np.float32)
    bo = np.asarray(bo, np.float32)

    tri = np.triu(np.ones((P, P), np.float32)).astype(BF16)

    in_maps = []
    for b in range(X.shape[0]):
        xt = np.ascontiguousarray(X[b].T).astype(BF16).reshape(KCH, P, seq)
        for hg in range(2):
            sl = slice(hg * DH, (hg + 1) * DH)
            in_maps.append(
                {
                    "xt": xt,
                    "wq": np.ascontiguousarray(Wq[:, sl]).astype(BF16).reshape(KCH, P, DH),
                    "wk": np.ascontiguousarray(Wk[:, sl]).astype(BF16).reshape(KCH, P, DH),
                    "wv": np.ascontiguousarray(Wv[:, sl]).astype(BF16).reshape(KCH, P, DH),
                    "wo": np.ascontiguousarray(Wo[sl, :]).astype(BF16).reshape(3, P, DIN),
                    "tri": tri,
                }
            )
    return in_maps


_built = None


def _get_built():
    global _built
    if _built is None:
        _built = build()
    return _built


def run(inputs, trace=False):
    from concourse.bass_utils import run_bass_kernel_spmd

    nc = _get_built()
    in_maps = make_in_maps(**inputs)
    res = run_bass_kernel_spmd(nc, in_maps, list(range(8)), trace=trace)
    # per-core output is stored transposed as [6, 128, seq] = out.T chunked
    parts = [
        np.asarray(r["out"]).astype(np.float32).reshape(DIN, -1).T
        for r in res.results
    ]
    bo = np.asarray(inputs["bo"], np.float32)
    out = np.stack(
        [parts[2 * b] + parts[2 * b + 1] + bo for b in range(len(parts) // 2)]
    )
    return out, res


def kernel(X, Wq, Wk, Wv, Wo, bo):
    out, _ = run(dict(X=X, Wq=Wq, Wk=Wk, Wv=Wv, Wo=Wo, bo=bo))
    return out


# revision 12
# speedup vs baseline: 1.1181x; 1.0224x over previous
"""Trainium2 Bass kernel for causal multi-head attention.

Problem: nn_MultiHeadAttention (B=4, N=2048, D=768, H=12, dh=64), fp32 I/O.

Sharding: 8 cores = 4 batches x 2 head-groups (6 heads each).  Each core
computes QKV projections for its 6 heads, causal softmax attention, and a
partial output projection (its heads' rows of Wo).  The two partials per
batch are summed on the host (tensor-parallel reduce), which also adds the
output bias; the kernel writes its partial out^T in bf16 (halves output
DMA traffic).

Per-core layout strategy (all matmuls in bf16, fp32 accumulate):
  - X^T is prepared host-side: xt[c,p,n] = X[n, 128c+p] (bf16), streamed
    as 12 half-chunks round-robin over the sync/vector/gpsimd/tensor DMA
    queues so the first QK matmuls start ~3us in; a short burst of warmup
    matmuls keeps the PE HAM clock-gate warm through the load.
  - Q^T, K^T computed as [384, N] (d_out on partitions); pair-0's first
    half is kc-interleaved with the xt DMA stream, later pairs run as
    fill work inside the attention loop.
  - V computed in natural [N, 64h] layout, extended with a ones column per
    head so the context matmul also produces the softmax denominators.
  - scores^T tiles [k=128, 2 heads, q=512] in PSUM (two row-tiled
    concurrent matmuls), exp on ScalarE (scale=1/8 fused), causal diagonal
    masked by a triangle multiply.
  - ctx^T accumulated in PSUM over k chunks; row 64 = sum_k exp (denom).
  - denominators -> packed [128,8] reciprocal -> DRAM -> partition-
    broadcast DMAs riding the vector/gpsimd queues (keeps the sync queue
    shallow) -> per-q inverse scale applied on DVE.
  - out^T = Wo_chunk^T @ cn per 128-row chunk; emission of each q-block's
    output projection is delayed by one attention block so its pair-2
    matmul never head-of-line-blocks the in-order PE queue; the final
    q-block runs a two-phase projection (pairs 0/1 accumulate while the
    last denominator chain resolves, pair 2 finishes after) with the
    PSUM->SBUF casts split across DVE/ScalarE and the output DMAs spread
    over three queues.
"""

import sys

sys.path.insert(0, "/opt/trn_rl_repo")

import numpy as np
import ml_dtypes

BF16 = ml_dtypes.bfloat16

P = 128
DIN = 768
DH = 384  # per-core output cols of Wq/Wk/Wv (6 heads x 64)
NH = 6  # heads per core
KCH = 6  # d_in chunks (768/128)
QW = 512  # q block width


def build(seq=2048, n_wchunks=3):
    """Build the SPMD single-core program.  seq parameterized for sim tests."""
    import concourse.mybir as mybir
    import concourse.tile as tile
    from concourse import bacc
    from contextlib import ExitStack

    f32 = mybir.dt.float32
    bf16 = mybir.dt.bfloat16
    EXP = mybir.ActivationFunctionType.Exp
    COPY = mybir.ActivationFunctionType.Copy

    nqb = seq // QW  # q blocks of 512
    nkc = seq // P  # k chunks of 128
    HP = 3  # head pairs
    HSEQ = seq // 2

    nc = bacc.Bacc(None, target_bir_lowering=False, debug=False)

    xt_d = nc.dram_tensor("xt", [KCH, P, seq], bf16, kind="ExternalInput")
    wq_d = nc.dram_tensor("wq", [KCH, P, DH], bf16, kind="ExternalInput")
    wk_d = nc.dram_tensor("wk", [KCH, P, DH], bf16, kind="ExternalInput")
    wv_d = nc.dram_tensor("wv", [KCH, P, DH], bf16, kind="ExternalInput")
    wo_d = nc.dram_tensor("wo", [n_wchunks, P, DIN], bf16, kind="ExternalInput")
    tri_d = nc.dram_tensor("tri", [P, P], bf16, kind="ExternalInput")
    # output is stored transposed: out[e_chunk, e_p, q] = full_out[q, 128*e_chunk+e_p]
    out_d = nc.dram_tensor("out", [DIN // P, P, seq], bf16, kind="ExternalOutput")
    inv_d = nc.dram_tensor("inv_scratch", [HP, nqb, 2, QW], f32)

    with tile.TileContext(nc) as tc, ExitStack() as ctx:
        const = ctx.enter_context(tc.tile_pool(name="const", bufs=1))
        io = ctx.enter_context(tc.tile_pool(name="io", bufs=1))
        expp = ctx.enter_context(tc.tile_pool(name="expp", bufs=8))
        crawp = ctx.enter_context(tc.tile_pool(name="crawp", bufs=3))
        smallp = ctx.enter_context(tc.tile_pool(name="smallp", bufs=4))
        invbp = ctx.enter_context(tc.tile_pool(name="invbp", bufs=4))
        outp = ctx.enter_context(tc.tile_pool(name="outp", bufs=3))
        ps = ctx.enter_context(tc.tile_pool(name="ps", bufs=3, space="PSUM"))
        cxps = ctx.enter_context(tc.tile_pool(name="cxps", bufs=1, space="PSUM"))

        # ---------------- persistent inputs ----------------
        xt = const.tile([P, KCH, seq], bf16, name="xt_sb")
        wq = const.tile([P, KCH, DH], bf16, name="wq_sb")
        wk = const.tile([P, KCH, DH], bf16, name="wk_sb")
        wv = const.tile([P, KCH, DH], bf16, name="wv_sb")
        wo = const.tile([P, n_wchunks, DIN], bf16, name="wo_sb")
        tri = const.tile([P, P], bf16, name="tri_sb")

        # tri leads on sync (warmup matmul source), then xt streams as
        # half-chunks: sync carries chunks 0-2, gpsimd 3-5, first halves
        # before second halves so the upfront QK matmuls (which consume
        # first halves in arrival order) start as soon as possible.
        # pair-0 weight slices lead on the scalar queue.
        nc.sync.dma_start(tri[:], tri_d[:])
        for h in range(2):
            sl = slice(h * HSEQ, (h + 1) * HSEQ)
            for c in range(KCH // 2):
                nc.sync.dma_start(xt[:, c, sl], xt_d[c][:, sl])
                nc.gpsimd.dma_start(
                    xt[:, c + KCH // 2, sl], xt_d[c + KCH // 2][:, sl]
                )
        nc.scalar.dma_start(
            wq[:, :, 0:P], wq_d[:, :, 0:P].rearrange("c p x -> p c x")
        )
        nc.scalar.dma_start(
            wk[:, :, 0:P], wk_d[:, :, 0:P].rearrange("c p x -> p c x")
        )
        for c in range(KCH):
            nc.scalar.dma_start(wv[:, c, :], wv_d[c])
        nc.scalar.dma_start(
            wq[:, :, P:DH], wq_d[:, :, P:DH].rearrange("c p x -> p c x")
        )
        nc.scalar.dma_start(
            wk[:, :, P:DH], wk_d[:, :, P:DH].rearrange("c p x -> p c x")
        )
        for c in range(n_wchunks):
            nc.scalar.dma_start(wo[:, c, :], wo_d[c])
        # (sync: tri + chunks 0-2; gpsimd: chunks 3-5; half-chunk arrival
        # order is therefore 0,3,1,4,2,5 — KC_ORDER below matches.)

        # persistent activations
        qt = io.tile([P, HP, seq], bf16, name="qt_sb")
        kt = io.tile([P, HP, seq], bf16, name="kt_sb")
        vx = io.tile([P, nkc, NH, 65], bf16, name="vx_sb")
        cn = io.tile([P, HP, seq], bf16, name="cn_sb")
        ones = io.tile([65, 64], bf16, name="ones_sb")
        nc.vector.memset(vx[:, :, :, 64:65], 1.0)
        nc.vector.memset(ones[:], 1.0)

        # ---------------- warmup: keep HAM busy during the load ----------
        warm = ps.tile([P, 2, QW], f32, tag="quad", name="warm")
        for _ in range(16):
            nc.tensor.matmul(
                warm[:, 0, 0:P],
                lhsT=tri[:],
                rhs=tri[:],
                start=True,
                stop=True,
            )

        # xt half-chunks arrive interleaved across the two queues: chunk
        # order 0,3,1,4,2,5 — consume in that order.
        KC_ORDER = [c for pair_ in zip(range(KCH // 2), range(KCH // 2, KCH)) for c in pair_]

        def qk_upfront(nbs):
            """Pair-0 Q^T and K^T over two n blocks, kc-interleaved so the
            matmul stream follows the xt DMA arrivals."""
            nbs = tuple(nb for nb in nbs if nb < nqb)
            if not nbs:
                return
            ptq = ps.tile([P, 2, QW], f32, tag="quad", name="ptq")
            ptk = ps.tile([P, 2, QW], f32, tag="quad", name="ptk")
            for ki, kc in enumerate(KC_ORDER):
                for pt, wt in ((ptq, wq), (ptk, wk)):
                    for r, nb in enumerate(nbs):
                        nc.tensor.matmul(
                            pt[:, r, :],
                            lhsT=wt[:, kc, 0:P],
                            rhs=xt[:, kc, nb * QW : (nb + 1) * QW],
                            start=(ki == 0),
                            stop=(ki == KCH - 1),
                        )
            for pt, dst in ((ptq, qt), (ptk, kt)):
                for r, nb in enumerate(nbs):
                    nc.vector.tensor_copy(dst[:, 0, nb * QW : (nb + 1) * QW], pt[:, r, :])

        def qk_quarter(pair, quarter):
            """Project one quarter of pair's Q^T/K^T: one weight chunk reused
            across two 512-wide n blocks (kc-outer keeps LDWEIGHTS warm).
            Yields after each matmul so the caller can interleave."""
            wt, dst = (wq, qt) if quarter < 2 else (wk, kt)
            nbs = (0, 1) if quarter % 2 == 0 else (2, 3)
            nbs = tuple(nb for nb in nbs if nb < nqb)
            if not nbs:
                return
            pt = ps.tile([P, 2, QW], f32, tag="quad", name="pt")
            for kc in range(KCH):
                for r, nb in enumerate(nbs):
                    nc.tensor.matmul(
                        pt[:, r, :],
                        lhsT=wt[:, kc, pair * P : (pair + 1) * P],
                        rhs=xt[:, kc, nb * QW : (nb + 1) * QW],
                        start=(kc == 0),
                        stop=(kc == KCH - 1),
                    )
                    yield
            for r, nb in enumerate(nbs):
                nc.vector.tensor_copy(dst[:, pair, nb * QW : (nb + 1) * QW], pt[:, r, :])

        def v_chunk(nb):
            """Yields after each matmul so the caller can interleave."""
            pt = ps.tile([P, 2, QW], f32, tag="quad", name="pt")
            for kc in range(KCH):
                nc.tensor.matmul(
                    pt[:, 0, :DH],
                    lhsT=xt[:, kc, nb * P : (nb + 1) * P],
                    rhs=wv[:, kc, :],
                    start=(kc == 0),
                    stop=(kc == KCH - 1),
                )
                yield
            nc.vector.tensor_copy(
                vx[:, nb, :, 0:64],
                pt[:, 0, :DH].rearrange("p (h d) -> p h d", d=64),
            )

        def out_proj_t(e, qb):
            """Transposed output projection: out^T[e-chunk, q-block] =
            Wo_chunk^T @ cn.  Yields after each matmul so the caller can
            interleave; output DMAs alternate queues."""
            qsl = slice(qb * QW, (qb + 1) * QW)
            op = ps.tile([P, 2, QW], f32, tag="quad", name="op")
            for c in range(n_wchunks):
                nc.tensor.matmul(
                    op[:, 0, :],
                    lhsT=wo[:, c, e * P : (e + 1) * P],
                    rhs=cn[:, c, qsl],
                    start=(c == 0),
                    stop=(c == n_wchunks - 1),
                )
                yield
            ob = outp.tile([P, QW], bf16, name="ob")
            nc.vector.tensor_copy(ob[:], op[:, 0, :])
            (nc.gpsimd if e % 2 else nc.sync).dma_start(out_d[e, :, qsl], ob[:])

        class FillQueue:
            """Queue of instruction generators, driven a few matmuls at a
            time from inside the attention loop to fill PE bubbles."""

            def __init__(self):
                self.gens = []

            def add(self, gen):
                self.gens.append(gen)

            def step(self, n):
                while n > 0 and self.gens:
                    try:
                        next(self.gens[0])
                        n -= 1
                    except StopIteration:
                        self.gens.pop(0)

            def drain(self):
                while self.gens:
                    self.step(1000)

        def attention_qblock(pair, i, fq, steps=(1, 2), last=False):
            """One 512-wide q block of causal attention for a head pair.
            fq: FillQueue driven mid-loop to fill PE bubbles.  Returns a
            deferred finish closure when `last` (the kernel tail overlaps
            it with the final output projection)."""
            nj = 4 * i + 4  # active k chunks
            qsl = slice(i * QW, (i + 1) * QW)
            cx = cxps.tile([65, 2, QW], f32, name="cx")
            pend = []  # (j, exp tile) awaiting ctx matmuls

            def ctx_mms(j, et):
                d = j - 4 * i
                off = P * d if d > 0 else 0
                for hh in (0, 1):
                    nc.tensor.matmul(
                        cx[:, hh, off:],
                        lhsT=vx[:, j, 2 * pair + hh, :],
                        rhs=et[:, hh, off:],
                        start=(j == 0),
                        stop=(j == nj - 1),
                    )

            for j in range(nj):
                d = j - 4 * i
                off = P * d if d > 0 else 0
                sc = ps.tile([P, 2, QW], f32, tag="quad", name="sc")
                for hh in (0, 1):
                    base = 64 * hh
                    nc.tensor.matmul(
                        sc[:, hh, off:],
                        lhsT=kt[base : base + 64, pair, j * P : (j + 1) * P],
                        rhs=qt[base : base + 64, pair, i * QW + off : (i + 1) * QW],
                        start=True,
                        stop=True,
                        tile_position=(base, 0),
                    )
                et = expp.tile([P, 2, QW], bf16, name="et")
                if off:
                    nc.scalar.activation(
                        et[:, :, off:], sc[:, :, off:], EXP, scale=0.125
                    )
                else:
                    nc.scalar.activation(et[:], sc[:], EXP, scale=0.125)
                if d >= 0:
                    for hh in (0, 1):
                        nc.vector.tensor_mul(
                            et[:, hh, off : off + P], et[:, hh, off : off + P], tri[:]
                        )
                if len(pend) >= 2:
                    ctx_mms(*pend.pop(0))
                pend.append((j, et))
                fq.step(steps[1] if d >= 0 else steps[0])
            for args in pend:
                ctx_mms(*args)

            # ctx + denominators out of PSUM
            cr = crawp.tile([65, 2, QW], f32, name="cr")
            nc.vector.tensor_copy(cr[:], cx[:])

            # denominators -> packed reciprocal
            sp = smallp.tile([P, 8], f32, tag="sp", name="sp")
            nc.sync.dma_start(sp[:], cr[64:65, :, :])
            if last:
                # Final block: short on-chip chain; the PE ones-broadcast +
                # normalize is deferred so the tail output projection's
                # pair-0/1 matmuls cover the reciprocal round trip.
                ipb = smallp.tile([P, 8], bf16, tag="ip", name="ipb")
                with nc.allow_low_precision(reason="inv denom broadcast bf16"):
                    nc.vector.reciprocal(ipb[:], sp[:])
                rb = smallp.tile([65, 2, QW], bf16, tag="rb", name="rb")
                nc.sync.dma_start(rb[64:65, :, :], ipb[:])

                def finish():
                    for hh in (0, 1):
                        nc.tensor.matmul(
                            cx[0:64, hh, :],
                            lhsT=ones[64:65, :],
                            rhs=rb[64:65, hh, :],
                            start=True,
                            stop=True,
                        )
                    tt = smallp.tile([64, QW], bf16, tag="tt", name="tt")
                    nc.vector.tensor_mul(tt[:], cr[0:64, 1, :], cx[0:64, 1, :])
                    nc.sync.dma_start(cn[64:128, pair, qsl], tt[:])
                    nc.vector.tensor_mul(
                        cn[0:64, pair, qsl], cr[0:64, 0, :], cx[0:64, 0, :]
                    )

                return finish

            ip = smallp.tile([P, 8], f32, tag="ip", name="ip")
            nc.vector.reciprocal(ip[:], sp[:])
            nc.sync.dma_start(inv_d[pair, i], ip[:])

            # inverse broadcasts + normalize; the two broadcasts split
            # across the sync and gpsimd queues.
            for hh in (0, 1):
                ib = invbp.tile([64, QW], f32, name="ib")
                (nc.sync if hh == 0 else nc.gpsimd).dma_start(
                    ib[:], inv_d[pair, i, hh : hh + 1, :].broadcast_to((64, QW))
                )
                if hh == 0:
                    nc.vector.tensor_mul(cn[0:64, pair, qsl], cr[0:64, 0, :], ib[:])
                else:
                    tt = smallp.tile([64, QW], bf16, tag="tt", name="tt")
                    nc.vector.tensor_mul(tt[:], cr[0:64, 1, :], ib[:])
                    (nc.sync if i % 2 else nc.gpsimd).dma_start(
                        cn[64:128, pair, qsl], tt[:]
                    )
            return None

        # ---------------- schedule ----------------
        # DMA-paced upfront QK for pair 0, then dense quarters + V, then
        # attention with later pairs' projections / the output projection
        # interleaved as fine-grained fill work.  The fill queue persists
        # across pair boundaries (no drain bursts).
        qk_upfront((0, 1))
        # V chunks 0..7 touch only the first xt halves — run them while the
        # second halves stream in, then the second upfront pass.
        for nb in range(min(nkc, 8)):
            for _ in v_chunk(nb):
                pass
        qk_upfront((2, 3))
        for nb in range(min(nkc, 8), nkc):
            for _ in v_chunk(nb):
                pass
        fq = FillQueue()
        finish_last = None
        for pair in range(HP):
            if pair < HP - 1:
                for quarter in range(4):
                    fq.add(qk_quarter(pair + 1, quarter))
            for i in range(nqb):
                last = pair == HP - 1 and i == nqb - 1
                r = attention_qblock(
                    pair, i, fq, steps=(1, 2) if pair < HP - 1 else (2, 2), last=last
                )
                if r is not None:
                    finish_last = r
                # delay each q block's output projection by one block so its
                # pair-2 matmul never enters the PE queue before the block's
                # normalize chain is emitted and well underway.
                if pair == HP - 1 and i >= 1:
                    for e in range(DIN // P):
                        fq.add(out_proj_t(e, i - 1))

        # ---------------- tail: final q block's output projection --------
        qb = nqb - 1
        qsl = slice(qb * QW, (qb + 1) * QW)
        ne = DIN // P
        fq.drain()
        # phase A: Wo pairs 0/1 accumulate into three double-slot PSUM
        # tiles while the deferred denominator chain resolves off-PE.
        ops = [
            ps.tile([P, 2, QW], f32, tag="quad", name=f"opt{t}")
            for t in range((ne + 1) // 2)
        ]
        for c in range(n_wchunks - 1):
            for e in range(ne):
                nc.tensor.matmul(
                    ops[e // 2][:, e % 2, :],
                    lhsT=wo[:, c, e * P : (e + 1) * P],
                    rhs=cn[:, c, qsl],
                    start=(c == 0),
                    stop=False,
                )
        if finish_last is not None:
            finish_last()
        # phase B: pair-2 contribution, then casts split across DVE and
        # ScalarE and output DMAs spread over three queues.
        for e in range(ne):
            nc.tensor.matmul(
                ops[e // 2][:, e % 2, :],
                lhsT=wo[:, n_wchunks - 1, e * P : (e + 1) * P],
                rhs=cn[:, n_wchunks - 1, qsl],
                start=False,
                stop=True,
            )
        for t in range((ne + 1) // 2):
            ob2 = outp.tile([P, 2, QW], bf16, name="ob2")
            if t % 2 == 1:
                nc.scalar.activation(ob2[:], ops[t][:], COPY)
            else:
                nc.vector.tensor_copy(ob2[:], ops[t][:])
            for k in range(2):
                e = 2 * t + k
                if e < ne:
                    [nc.gpsimd, nc.sync, nc.scalar][e % 3].dma_start(
                        out_d[e, :, qsl], ob2[:, k, :]
                    )

    nc.compile()
    return nc


def make_in_maps(X, Wq, Wk, Wv, Wo, bo=None, seq=2048):
    """Shard full inputs into the 8 per-core input maps."""
    X = np.asarray(X, np.float32)
    Wq = np.asarray(Wq, np.float32)
    Wk = np.asarray(Wk, np.float32)
    Wv = np.asarray(Wv, np.float32)
    Wo = np.asarray(Wo, np.float32)
    bo = np.asarray(bo, np.float32)

    tri = np.triu(np.ones((P, P), np.float32)).astype(BF16)

    in_maps = []
    for b in range(X.shape[0]):
        xt = np.ascontiguousarray(X[b].T).astype(BF16).reshape(KCH, P, seq)
        for hg in range(2):
            sl = slice(hg * DH, (hg + 1) * DH)
            in_maps.append(
                {
                    "xt": xt,
                    "wq": np.ascontiguousarray(Wq[:, sl]).astype(BF16).reshape(KCH, P, DH),
                    "wk": np.ascontiguousarray(Wk[:, sl]).astype(BF16).reshape(KCH, P, DH),
                    "wv": np.ascontiguousarray(Wv[:, sl]).astype(BF16).reshape(KCH, P, DH),
                    "wo": np.ascontiguousarray(Wo[sl, :]).astype(BF16).reshape(3, P, DIN),
                    "tri": tri,
                }
            )
    return in_maps


_built = None


def _get_built():
    global _built
    if _built is None:
        _built = build()
    return _built


def run(inputs, trace=False):
    from concourse.bass_utils import run_bass_kernel_spmd

    nc = _get_built()
    in_maps = make_in_maps(**inputs)
    res = run_bass_kernel_spmd(nc, in_maps, list(range(8)), trace=trace)
    # per-core output is stored transposed as [6, 128, seq] = out.T chunked
    parts = [
        np.asarray(r["out"]).astype(np.float32).reshape(DIN, -1).T
        for r in res.results
    ]
    bo = np.asarray(inputs["bo"], np.float32)
    out = np.stack(
        [parts[2 * b] + parts[2 * b + 1] + bo for b in range(len(parts) // 2)]
    )
    return out, res


def kernel(X, Wq, Wk, Wv, Wo, bo):
    out, _ = run(dict(X=X, Wq=Wq, Wk=Wk, Wv=Wv, Wo=Wo, bo=bo))
    return out
